# revision 14
# baseline (speedup 1.0000x reference)
"""EnhancedEMA Trainium2 kernel.

Math: the reference computes, per batch b:
  m_b   = mean(x[b])                                  (scalar)
  h1    = relu(m_b * rowsum(w1) + b1)                 ([C2]; C2 = T/4 = 512)
  tw    = softmax(h1 @ w2.T + b2)                     ([T])
  a_t   = position_alpha[t] * tw[t]                   (per (b,t) scalar)
  s_0   = x[b,0]; s_t = a_t x_t + (1-a_t) s_{t-1}     (EMA over T, vector over C)

The EMA coefficient is scalar per (b,t), so the recurrence linearizes:
  c_t = prod_{j=1..t}(1-a_j)  (c_0=1),  g_t = a_t/c_t
  s_t = c_t * (x_0 + Q_t),  Q_t = sum_{k=1..t} g_k x_k
Since sum_t a_t = 0.3 (softmax * 0.3), c_t in [~0.7, 1] -> numerically safe.

All PE matmuls run in bf16: the stationary matrices (upper-triangular ones U,
all-ones) are exactly representable, and the Q-path data terms are ~1.5e-4*x
so bf16 rounding is ~6e-7 absolute. The dominant x_0 term bypasses the matmul
path entirely (kept in f32, broadcast across partitions via GpSimd), giving
~3e-4 overall relative error. x is uploaded as bf16 (halves DMA-in traffic).

Device layout (per core, BPC=4 batches): column layout [t_lo=128 partitions,
(tb, ...) free]. The cumsum over T runs as triangular-ones matmuls per
128-block on the PE, with the inter-block carry injected by adding the
running column-sum (a second all-ones matmul, kept in PSUM) into row 0 of the
next block's input. cumprod(1-a) is done in log space with the same
matmul-cumsum trick.
"""

import os
import numpy as np
import ml_dtypes

B, T, C = 32, 2048, 512
NCORES = 8
BPC = B // NCORES        # batches per core
NTB = T // 128           # 16 t-blocks
C2 = T // 4              # 512 hidden
NJB = C2 // 128          # 4 j-blocks

_CACHE = {}
LAST_RESULTS = None


def _build_bass():
    import concourse.bass as bass
    import concourse.bacc as bacc
    import concourse.tile as tile
    from concourse import mybir
    from contextlib import ExitStack

    f32 = mybir.dt.float32
    bf16 = mybir.dt.bfloat16
    AF = mybir.ActivationFunctionType
    ALU = mybir.AluOpType
    AX = mybir.AxisListType

    nc = bacc.Bacc("TRN2", target_bir_lowering=False)

    x_d = nc.dram_tensor("x", [BPC, T, C], bf16, kind="ExternalInput")
    x0_d = nc.dram_tensor("x0", [BPC, C], f32, kind="ExternalInput")
    w2t_d = nc.dram_tensor("w2t", [NJB, 128, T], bf16, kind="ExternalInput")
    prm_d = nc.dram_tensor("prm", [128, 2 * NJB + 2 * NTB], f32,
                           kind="ExternalInput")
    u_d = nc.dram_tensor("ut", [128, 128], bf16, kind="ExternalInput")
    y_d = nc.dram_tensor("y", [BPC, T, C], f32, kind="ExternalOutput")

    with ExitStack() as ctx:
        tc = ctx.enter_context(tile.TileContext(nc))
        consts = ctx.enter_context(tc.tile_pool(name="consts", bufs=1))
        xpool = ctx.enter_context(tc.tile_pool(name="xpool", bufs=BPC))
        xrawpool = ctx.enter_context(tc.tile_pool(name="xrawpool", bufs=BPC))
        zpool = ctx.enter_context(tc.tile_pool(name="zpool", bufs=3))
        cxpool = ctx.enter_context(tc.tile_pool(name="cxpool", bufs=3))
        ypool = ctx.enter_context(tc.tile_pool(name="ypool", bufs=4))
        x0pool = ctx.enter_context(tc.tile_pool(name="x0pool", bufs=BPC))
        small = ctx.enter_context(tc.tile_pool(name="small", bufs=2))
        coef = ctx.enter_context(tc.tile_pool(name="coef", bufs=2))
        pscan = ctx.enter_context(tc.tile_pool(name="pscan", bufs=3, space="PSUM"))
        pcarry = ctx.enter_context(tc.tile_pool(name="pcarry", bufs=2, space="PSUM"))
        pmean = ctx.enter_context(tc.tile_pool(name="pmean", bufs=2, space="PSUM"))
        psmall = ctx.enter_context(tc.tile_pool(name="psmall", bufs=1, space="PSUM"))

        # ---- constants ----
        # DMA-produced tensors are copied ("absorbed") on DVE so downstream
        # consumers carry engine-sem waits only: a DMA-queue-sem wait must be
        # the ONLY wait on an instruction (HW encoding limit).
        NPRM = 2 * NJB + 2 * NTB
        u_raw = consts.tile([128, 128], bf16, name="u_raw")
        nc.sync.dma_start(out=u_raw, in_=u_d[:, :])
        u_sb = consts.tile([128, 128], bf16, name="u_sb")
        nc.vector.tensor_copy(u_sb, u_raw)
        w2t_raw = consts.tile([128, NJB, T], bf16, name="w2t_raw")
        nc.sync.dma_start(
            out=w2t_raw,
            in_=bass.AP(tensor=w2t_d[0].tensor, offset=0,
                        ap=[[T, 128], [128 * T, NJB], [1, T]]))
        w2t_sb = consts.tile([128, NJB, T], bf16, name="w2t_sb")
        nc.vector.tensor_copy(w2t_sb, w2t_raw)
        prm_raw = consts.tile([128, NPRM], f32, name="prm_raw")
        nc.sync.dma_start(out=prm_raw, in_=prm_d[:, :])
        prm_sb = consts.tile([128, NPRM], f32, name="prm_sb")
        nc.vector.tensor_copy(prm_sb, prm_raw)
        s1_sb = prm_sb[:, 0:NJB]
        b1_sb = prm_sb[:, NJB:2 * NJB]
        b2_sb = prm_sb[:, 2 * NJB:2 * NJB + NTB]
        pa_sb = prm_sb[:, 2 * NJB + NTB:2 * NJB + 2 * NTB]
        onesmat = consts.tile([128, 128], bf16, name="onesmat")
        nc.vector.memset(onesmat, 1.0)
        zeros16 = consts.tile([1, NTB], f32, name="zeros16")
        nc.vector.memset(zeros16, 0.0)

        for b in range(BPC):
            # ---- phase A: load batch (bf16), x0 row (f32), mean ----
            xraw = xrawpool.tile([128, NTB * C], bf16, name=f"xraw{b}",
                                 tag="xraw")
            xb = xpool.tile([128, NTB * C], bf16, name=f"xb{b}", tag="xb")
            for tb in range(NTB):
                nc.sync.dma_start(
                    out=xraw[:, tb * C:(tb + 1) * C],
                    in_=x_d[b, tb * 128:(tb + 1) * 128, :],
                )
                nc.vector.tensor_copy(xb[:, tb * C:(tb + 1) * C],
                                      xraw[:, tb * C:(tb + 1) * C])
            x0raw = small.tile([128, C], f32, name=f"x0raw{b}", tag="x0raw")
            x0_row = x0_d[b:b + 1, :]
            x0_bcast = bass.AP(tensor=x0_row.tensor, offset=x0_row.offset,
                               ap=[[0, 128], [1, C]])
            nc.sync.dma_start(out=x0raw, in_=x0_bcast)
            x0b = x0pool.tile([128, C], f32, name=f"x0b{b}", tag="x0b")
            nc.vector.tensor_copy(x0b, x0raw)

            pm = pmean.tile([128, C], f32, name=f"pm{b}", tag="pm")
            for tb in range(NTB):
                nc.tensor.matmul(pm, lhsT=onesmat,
                                 rhs=xb[:, tb * C:(tb + 1) * C],
                                 start=(tb == 0), stop=(tb == NTB - 1))
            # all pm rows equal the column sum -> free-reduce gives the total
            # replicated on every partition
            smr = small.tile([128, 1], f32, name=f"smr{b}", tag="smr")
            nc.vector.reduce_sum(out=smr, in_=pm, axis=AX.X)
            sm = small.tile([128, 1], f32, name=f"sm{b}", tag="sm")
            nc.scalar.mul(sm, smr, 1.0 / float(T * C))

            # ---- phase B: h1 = relu(m*s1+b1); logits; softmax -> a ----
            t1 = small.tile([128, NJB], f32, name=f"t1{b}", tag="t1")
            nc.vector.tensor_scalar_mul(t1, in0=s1_sb, scalar1=sm)
            nc.vector.tensor_add(t1, t1, b1_sb)
            h1t = small.tile([128, NJB], bf16, name=f"h1t{b}", tag="h1t")
            nc.scalar.activation(h1t, t1, AF.Relu)

            plb = psmall.tile([128, NTB], f32, name=f"plb{b}", tag="ps")
            for tb in range(NTB):
                for jb in range(NJB):
                    nc.tensor.matmul(
                        plb[:, tb:tb + 1],
                        lhsT=w2t_sb[:, jb, tb * 128:(tb + 1) * 128],
                        rhs=h1t[:, jb:jb + 1],
                        start=(jb == 0),
                        stop=(jb == NJB - 1),
                    )
            lg = coef.tile([128, NTB], f32, name=f"lg{b}", tag="lg")
            nc.vector.tensor_add(lg, plb, b2_sb)
            e = coef.tile([128, NTB], bf16, name=f"e{b}", tag="e")
            nc.scalar.activation(e, lg, AF.Exp)
            pse = psmall.tile([128, NTB], f32, name=f"pse{b}", tag="ps")
            nc.tensor.matmul(pse, lhsT=onesmat, rhs=e, start=True, stop=True)
            esumt = small.tile([128, 1], f32, name=f"esumt{b}", tag="esumt")
            nc.vector.reduce_sum(out=esumt, in_=pse, axis=AX.X)
            rb = small.tile([128, 1], f32, name=f"rb{b}", tag="rb")
            nc.vector.reciprocal(rb, esumt)
            acol = coef.tile([128, NTB], f32, name=f"acol{b}", tag="acol")
            nc.vector.tensor_scalar_mul(acol, in0=e, scalar1=rb)
            nc.vector.tensor_mul(acol, acol, pa_sb)

            # ---- phase C: c = exp(cumsum ln(1-a)), rc = 1/c, g = a*rc ----
            Lb = coef.tile([128, NTB], bf16, name=f"Lb{b}", tag="Lb")
            nc.scalar.activation(Lb, acol, AF.Ln, bias=1.0, scale=-1.0)
            nc.scalar.mul(Lb[0:1, 0:1], Lb[0:1, 0:1], 0.0)  # t=0: factor 1
            pT = psmall.tile([128, NTB], f32, name=f"pT{b}", tag="ps")
            nc.tensor.matmul(pT, lhsT=onesmat, rhs=Lb, start=True, stop=True)
            Tsb = small.tile([1, NTB], f32, name=f"Tsb{b}", tag="Tsb")
            nc.scalar.copy(Tsb, pT[0:1, :])
            stage = small.tile([128, NTB + 1], bf16, name=f"stage{b}", tag="stage")
            nc.vector.memset(stage, 0.0)
            nc.vector.tensor_tensor_scan(
                out=stage[0:1, 1:NTB + 1], data0=Tsb, data1=zeros16,
                initial=0.0, op0=ALU.add, op1=ALU.add,
            )
            pcs = psmall.tile([128, NTB], f32, name=f"pcs{b}", tag="ps")
            nc.tensor.matmul(pcs, lhsT=u_sb, rhs=Lb, start=True, stop=False)
            nc.tensor.matmul(pcs, lhsT=onesmat, rhs=stage[:, 0:NTB],
                             start=False, stop=True)
            ccol = coef.tile([128, NTB], f32, name=f"ccol{b}", tag="ccol")
            nc.scalar.activation(ccol, pcs, AF.Exp)
            rccol = coef.tile([128, NTB], f32, name=f"rccol{b}", tag="rccol")
            nc.scalar.activation(rccol, pcs, AF.Exp, scale=-1.0)
            gcol = coef.tile([128, NTB], f32, name=f"gcol{b}", tag="gcol")
            nc.vector.tensor_mul(gcol, acol, rccol)
            # t=0 contributes via the f32 x0 path, not Q: zero its weight
            nc.vector.memset(gcol[0:1, 0:1], 0.0)

            # ---- phase D: z = g*x (bf16); blocked cumsum Q with PSUM carry;
            #      y = c*Q + c*x0 ----
            pcprev = None
            for tb in range(NTB):
                z = zpool.tile([128, C], bf16, name=f"z{b}_{tb}", tag="z")
                nc.vector.tensor_scalar_mul(z, in0=xb[:, tb * C:(tb + 1) * C],
                                            scalar1=gcol[:, tb:tb + 1])
                if tb > 0:
                    # inject running total (column sum incl. prior carry)
                    nc.vector.tensor_add(z[0:1, :], z[0:1, :], pcprev[0:1, :])
                ps = pscan.tile([128, C], f32, name=f"psQ{b}_{tb}", tag="s")
                nc.tensor.matmul(ps, lhsT=u_sb, rhs=z, start=True, stop=True)
                if tb < NTB - 1:
                    pc = pcarry.tile([128, C], f32, name=f"pc{b}_{tb}", tag="pc")
                    nc.tensor.matmul(pc, lhsT=onesmat, rhs=z,
                                     start=True, stop=True)
                    pcprev = pc
                cx0 = cxpool.tile([128, C], f32, name=f"cx0{b}_{tb}", tag="cx0")
                nc.gpsimd.tensor_scalar_mul(cx0, in0=x0b,
                                            scalar1=ccol[:, tb:tb + 1])
                ysb = ypool.tile([128, C], f32, name=f"y{b}_{tb}", tag="y")
                nc.vector.scalar_tensor_tensor(
                    out=ysb, in0=ps, scalar=ccol[:, tb:tb + 1], in1=cx0,
                    op0=ALU.mult, op1=ALU.add)
                nc.gpsimd.dma_start(
                    out=y_d[b, tb * 128:(tb + 1) * 128, :], in_=ysb)

    nc.compile()
    return nc


def _get_nc():
    if "nc" not in _CACHE:
        _CACHE["nc"] = _build_bass()
    return _CACHE["nc"]


def kernel(x, position_alpha, w1, b1, w2, b2):
    global LAST_RESULTS
    from concourse.bass_utils import run_bass_kernel_spmd

    x = np.asarray(x, dtype=np.float32)
    position_alpha = np.asarray(position_alpha, dtype=np.float32)
    w1 = np.asarray(w1, dtype=np.float32)
    b1 = np.asarray(b1, dtype=np.float32)
    w2 = np.asarray(w2, dtype=np.float32)
    b2 = np.asarray(b2, dtype=np.float32)

    # host-side parameter prep (layout only / trivial reductions)
    s1 = w1.sum(axis=1)                                   # [C2]
    s1r = s1.reshape(NJB, 128).T                          # [128, NJB]
    b1r = b1.reshape(NJB, 128).T
    b2r = b2.reshape(NTB, 128).T                          # [128, NTB]
    par = position_alpha.reshape(NTB, 128).T
    prm = np.ascontiguousarray(
        np.concatenate([s1r, b1r, b2r, par], axis=1))     # [128, 2NJB+2NTB]
    w2t = np.ascontiguousarray(w2.T.reshape(NJB, 128, T)).astype(ml_dtypes.bfloat16)
    ut = np.triu(np.ones((128, 128), dtype=np.float32)).astype(ml_dtypes.bfloat16)
    x_bf = x.astype(ml_dtypes.bfloat16)
    x0 = np.ascontiguousarray(x[:, 0, :])                 # [B, C] f32

    nc = _get_nc()
    in_maps = []
    for i in range(NCORES):
        in_maps.append({
            "x": np.ascontiguousarray(x_bf[i * BPC:(i + 1) * BPC]),
            "x0": np.ascontiguousarray(x0[i * BPC:(i + 1) * BPC]),
            "w2t": w2t, "prm": prm, "ut": ut,
        })
    res = run_bass_kernel_spmd(
        nc, in_maps, core_ids=list(range(NCORES)),
        trace=bool(int(os.environ.get("EMA_TRACE", "0"))),
    )
    LAST_RESULTS = res
    return np.concatenate([r["y"] for r in res.results], axis=0)


# revision 15
# speedup vs baseline: 3.3791x; 3.3791x over previous
"""EnhancedEMA Trainium2 kernel.

Math: the reference computes, per batch b:
  m_b   = mean(x[b])                                  (scalar)
  h1    = relu(m_b * rowsum(w1) + b1)                 ([C2]; C2 = T/4 = 512)
  tw    = softmax(h1 @ w2.T + b2)                     ([T])
  a_t   = position_alpha[t] * tw[t]                   (per (b,t) scalar)
  s_0   = x[b,0]; s_t = a_t x_t + (1-a_t) s_{t-1}     (EMA over T, vector over C)

The EMA coefficient is scalar per (b,t), so the recurrence linearizes:
  c_t = prod_{j=1..t}(1-a_j)  (c_0=1),  g_t = a_t/c_t
  s_t = c_t * (x_0 + Q_t),  Q_t = sum_{k=1..t} g_k x_k
Since sum_t a_t = 0.3 (softmax * 0.3), c_t in [~0.7, 1] -> numerically safe.

All PE matmuls run in bf16: the stationary matrices (upper-triangular ones U,
all-ones) are exactly representable, and the Q-path data terms are ~1.5e-4*x
so bf16 rounding is ~6e-7 absolute. The dominant x_0 term bypasses the matmul
path entirely (kept in f32, broadcast across partitions via GpSimd), giving
~3e-4 overall relative error. x is uploaded as bf16 (halves DMA-in traffic).

Device layout (per core, BPC=4 batches): column layout [t_lo=128 partitions,
(tb, ...) free]. The cumsum over T runs as triangular-ones matmuls per
128-block on the PE, with the inter-block carry injected by adding the
running column-sum (a second all-ones matmul, kept in PSUM) into row 0 of the
next block's input. cumprod(1-a) is done in log space with the same
matmul-cumsum trick.
"""

import os
import numpy as np
import ml_dtypes

B, T, C = 32, 2048, 512
NCORES = 8
BPC = B // NCORES        # batches per core
NTB = T // 128           # 16 t-blocks
C2 = T // 4              # 512 hidden
NJB = C2 // 128          # 4 j-blocks

_CACHE = {}
LAST_RESULTS = None


def _build_bass():
    import concourse.bass as bass
    import concourse.bacc as bacc
    import concourse.tile as tile
    from concourse import mybir
    from contextlib import ExitStack

    f32 = mybir.dt.float32
    bf16 = mybir.dt.bfloat16
    AF = mybir.ActivationFunctionType
    ALU = mybir.AluOpType
    AX = mybir.AxisListType

    nc = bacc.Bacc("TRN2", target_bir_lowering=False)

    x_d = nc.dram_tensor("x", [BPC, T, C], bf16, kind="ExternalInput")
    x0_d = nc.dram_tensor("x0", [BPC, C], f32, kind="ExternalInput")
    w2t_d = nc.dram_tensor("w2t", [NJB, 128, T], bf16, kind="ExternalInput")
    prm_d = nc.dram_tensor("prm", [128, 2 * NJB + 2 * NTB], f32,
                           kind="ExternalInput")
    u_d = nc.dram_tensor("ut", [128, 128], bf16, kind="ExternalInput")
    y_d = nc.dram_tensor("y", [BPC, T, C], f32, kind="ExternalOutput")

    with ExitStack() as ctx:
        tc = ctx.enter_context(tile.TileContext(nc))
        consts = ctx.enter_context(tc.tile_pool(name="consts", bufs=1))
        xpool = ctx.enter_context(tc.tile_pool(name="xpool", bufs=BPC))
        zpool = ctx.enter_context(tc.tile_pool(name="zpool", bufs=3))
        cxpool = ctx.enter_context(tc.tile_pool(name="cxpool", bufs=3))
        ypool = ctx.enter_context(tc.tile_pool(name="ypool", bufs=4))
        x0pool = ctx.enter_context(tc.tile_pool(name="x0pool", bufs=BPC))
        small = ctx.enter_context(tc.tile_pool(name="small", bufs=2))
        coef = ctx.enter_context(tc.tile_pool(name="coef", bufs=2))
        pscan = ctx.enter_context(tc.tile_pool(name="pscan", bufs=3, space="PSUM"))
        pcarry = ctx.enter_context(tc.tile_pool(name="pcarry", bufs=2, space="PSUM"))
        pmean = ctx.enter_context(tc.tile_pool(name="pmean", bufs=2, space="PSUM"))
        psmall = ctx.enter_context(tc.tile_pool(name="psmall", bufs=1, space="PSUM"))

        # ---- constants ----
        # DMA-produced tensors are copied ("absorbed") on DVE so downstream
        # consumers carry engine-sem waits only: a DMA-queue-sem wait must be
        # the ONLY wait on an instruction (HW encoding limit).
        NPRM = 2 * NJB + 2 * NTB
        u_sb = consts.tile([128, 128], bf16, name="u_sb")
        nc.sync.dma_start(out=u_sb, in_=u_d[:, :])
        w2t_sb = consts.tile([128, NJB, T], bf16, name="w2t_sb")
        nc.sync.dma_start(
            out=w2t_sb,
            in_=bass.AP(tensor=w2t_d[0].tensor, offset=0,
                        ap=[[T, 128], [128 * T, NJB], [1, T]]))
        prm_sb = consts.tile([128, NPRM], f32, name="prm_sb")
        nc.sync.dma_start(out=prm_sb, in_=prm_d[:, :])
        s1_sb = prm_sb[:, 0:NJB]
        b1_sb = prm_sb[:, NJB:2 * NJB]
        b2_sb = prm_sb[:, 2 * NJB:2 * NJB + NTB]
        pa_sb = prm_sb[:, 2 * NJB + NTB:2 * NJB + 2 * NTB]
        onesmat = consts.tile([128, 128], bf16, name="onesmat")
        nc.vector.memset(onesmat, 1.0)
        zeros16 = consts.tile([1, NTB], f32, name="zeros16")
        nc.vector.memset(zeros16, 0.0)

        for b in range(BPC):
            # ---- phase A: load batch (bf16), x0 row (f32), mean ----
            xb = xpool.tile([128, NTB * C], bf16, name=f"xb{b}", tag="xb")
            for tb in range(NTB):
                nc.sync.dma_start(
                    out=xb[:, tb * C:(tb + 1) * C],
                    in_=x_d[b, tb * 128:(tb + 1) * 128, :],
                )
            x0b = x0pool.tile([128, C], f32, name=f"x0b{b}", tag="x0b")
            x0_row = x0_d[b:b + 1, :]
            x0_bcast = bass.AP(tensor=x0_row.tensor, offset=x0_row.offset,
                               ap=[[0, 128], [1, C]])
            nc.sync.dma_start(out=x0b, in_=x0_bcast)

            pm = pmean.tile([128, C], f32, name=f"pm{b}", tag="pm")
            for tb in range(NTB):
                nc.tensor.matmul(pm, lhsT=onesmat,
                                 rhs=xb[:, tb * C:(tb + 1) * C],
                                 start=(tb == 0), stop=(tb == NTB - 1))
            # all pm rows equal the column sum -> free-reduce gives the total
            # replicated on every partition
            smr = small.tile([128, 1], f32, name=f"smr{b}", tag="smr")
            nc.vector.reduce_sum(out=smr, in_=pm, axis=AX.X)
            sm = small.tile([128, 1], f32, name=f"sm{b}", tag="sm")
            nc.scalar.mul(sm, smr, 1.0 / float(T * C))

            # ---- phase B: h1 = relu(m*s1+b1); logits; softmax -> a ----
            t1 = small.tile([128, NJB], f32, name=f"t1{b}", tag="t1")
            nc.vector.tensor_scalar_mul(t1, in0=s1_sb, scalar1=sm)
            nc.vector.tensor_add(t1, t1, b1_sb)
            h1t = small.tile([128, NJB], bf16, name=f"h1t{b}", tag="h1t")
            nc.scalar.activation(h1t, t1, AF.Relu)

            plb = psmall.tile([128, NTB], f32, name=f"plb{b}", tag="ps")
            for tb in range(NTB):
                for jb in range(NJB):
                    nc.tensor.matmul(
                        plb[:, tb:tb + 1],
                        lhsT=w2t_sb[:, jb, tb * 128:(tb + 1) * 128],
                        rhs=h1t[:, jb:jb + 1],
                        start=(jb == 0),
                        stop=(jb == NJB - 1),
                    )
            lg = coef.tile([128, NTB], f32, name=f"lg{b}", tag="lg")
            nc.vector.tensor_add(lg, plb, b2_sb)
            e = coef.tile([128, NTB], bf16, name=f"e{b}", tag="e")
            nc.scalar.activation(e, lg, AF.Exp)
            pse = psmall.tile([128, NTB], f32, name=f"pse{b}", tag="ps")
            nc.tensor.matmul(pse, lhsT=onesmat, rhs=e, start=True, stop=True)
            esumt = small.tile([128, 1], f32, name=f"esumt{b}", tag="esumt")
            nc.vector.reduce_sum(out=esumt, in_=pse, axis=AX.X)
            rb = small.tile([128, 1], f32, name=f"rb{b}", tag="rb")
            nc.vector.reciprocal(rb, esumt)
            acol = coef.tile([128, NTB], f32, name=f"acol{b}", tag="acol")
            nc.vector.tensor_scalar_mul(acol, in0=e, scalar1=rb)
            nc.vector.tensor_mul(acol, acol, pa_sb)

            # ---- phase C: c = exp(cumsum ln(1-a)), rc = 1/c, g = a*rc ----
            Lb = coef.tile([128, NTB], bf16, name=f"Lb{b}", tag="Lb")
            nc.scalar.activation(Lb, acol, AF.Ln, bias=1.0, scale=-1.0)
            nc.scalar.mul(Lb[0:1, 0:1], Lb[0:1, 0:1], 0.0)  # t=0: factor 1
            pT = psmall.tile([128, NTB], f32, name=f"pT{b}", tag="ps")
            nc.tensor.matmul(pT, lhsT=onesmat, rhs=Lb, start=True, stop=True)
            Tsb = small.tile([1, NTB], f32, name=f"Tsb{b}", tag="Tsb")
            nc.scalar.copy(Tsb, pT[0:1, :])
            stage = small.tile([128, NTB + 1], bf16, name=f"stage{b}", tag="stage")
            nc.vector.memset(stage, 0.0)
            nc.vector.tensor_tensor_scan(
                out=stage[0:1, 1:NTB + 1], data0=Tsb, data1=zeros16,
                initial=0.0, op0=ALU.add, op1=ALU.add,
            )
            pcs = psmall.tile([128, NTB], f32, name=f"pcs{b}", tag="ps")
            nc.tensor.matmul(pcs, lhsT=u_sb, rhs=Lb, start=True, stop=False)
            nc.tensor.matmul(pcs, lhsT=onesmat, rhs=stage[:, 0:NTB],
                             start=False, stop=True)
            ccol = coef.tile([128, NTB], f32, name=f"ccol{b}", tag="ccol")
            nc.scalar.activation(ccol, pcs, AF.Exp)
            rccol = coef.tile([128, NTB], f32, name=f"rccol{b}", tag="rccol")
            nc.scalar.activation(rccol, pcs, AF.Exp, scale=-1.0)
            gcol = coef.tile([128, NTB], f32, name=f"gcol{b}", tag="gcol")
            nc.vector.tensor_mul(gcol, acol, rccol)
            # t=0 contributes via the f32 x0 path, not Q: zero its weight
            nc.vector.memset(gcol[0:1, 0:1], 0.0)

            # ---- phase D: z = g*x (bf16); blocked cumsum Q with PSUM carry;
            #      y = c*Q + c*x0 ----
            pcprev = None
            for tb in range(NTB):
                z = zpool.tile([128, C], bf16, name=f"z{b}_{tb}", tag="z")
                nc.scalar.activation(z, xb[:, tb * C:(tb + 1) * C], AF.Copy,
                                     scale=gcol[:, tb:tb + 1])
                if tb > 0:
                    # inject running total (column sum incl. prior carry)
                    nc.vector.tensor_add(z[0:1, :], z[0:1, :], pcprev[0:1, :])
                ps = pscan.tile([128, C], f32, name=f"psQ{b}_{tb}", tag="s")
                nc.tensor.matmul(ps, lhsT=u_sb, rhs=z, start=True, stop=True)
                if tb < NTB - 1:
                    pc = pcarry.tile([128, C], f32, name=f"pc{b}_{tb}", tag="pc")
                    nc.tensor.matmul(pc, lhsT=onesmat, rhs=z,
                                     start=True, stop=True)
                    pcprev = pc
                cx0 = cxpool.tile([128, C], f32, name=f"cx0{b}_{tb}", tag="cx0")
                nc.scalar.activation(cx0, x0b, AF.Copy,
                                     scale=ccol[:, tb:tb + 1])
                ysb = ypool.tile([128, C], f32, name=f"y{b}_{tb}", tag="y")
                nc.vector.scalar_tensor_tensor(
                    out=ysb, in0=ps, scalar=ccol[:, tb:tb + 1], in1=cx0,
                    op0=ALU.mult, op1=ALU.add)
                nc.sync.dma_start(
                    out=y_d[b, tb * 128:(tb + 1) * 128, :], in_=ysb)

    nc.compile()
    return nc


def _get_nc():
    if "nc" not in _CACHE:
        _CACHE["nc"] = _build_bass()
    return _CACHE["nc"]


def kernel(x, position_alpha, w1, b1, w2, b2):
    global LAST_RESULTS
    from concourse.bass_utils import run_bass_kernel_spmd

    x = np.asarray(x, dtype=np.float32)
    position_alpha = np.asarray(position_alpha, dtype=np.float32)
    w1 = np.asarray(w1, dtype=np.float32)
    b1 = np.asarray(b1, dtype=np.float32)
    w2 = np.asarray(w2, dtype=np.float32)
    b2 = np.asarray(b2, dtype=np.float32)

    # host-side parameter prep (layout only / trivial reductions)
    s1 = w1.sum(axis=1)                                   # [C2]
    s1r = s1.reshape(NJB, 128).T                          # [128, NJB]
    b1r = b1.reshape(NJB, 128).T
    b2r = b2.reshape(NTB, 128).T                          # [128, NTB]
    par = position_alpha.reshape(NTB, 128).T
    prm = np.ascontiguousarray(
        np.concatenate([s1r, b1r, b2r, par], axis=1))     # [128, 2NJB+2NTB]
    w2t = np.ascontiguousarray(w2.T.reshape(NJB, 128, T)).astype(ml_dtypes.bfloat16)
    ut = np.triu(np.ones((128, 128), dtype=np.float32)).astype(ml_dtypes.bfloat16)
    x_bf = x.astype(ml_dtypes.bfloat16)
    x0 = np.ascontiguousarray(x[:, 0, :])                 # [B, C] f32

    nc = _get_nc()
    in_maps = []
    for i in range(NCORES):
        in_maps.append({
            "x": np.ascontiguousarray(x_bf[i * BPC:(i + 1) * BPC]),
            "x0": np.ascontiguousarray(x0[i * BPC:(i + 1) * BPC]),
            "w2t": w2t, "prm": prm, "ut": ut,
        })
    res = run_bass_kernel_spmd(
        nc, in_maps, core_ids=list(range(NCORES)),
        trace=bool(int(os.environ.get("EMA_TRACE", "0"))),
    )
    LAST_RESULTS = res
    return np.concatenate([r["y"] for r in res.results], axis=0)
